# revision 52
# baseline (speedup 1.0000x reference)
"""Causal GQA attention (B=4, S=1024, H=16 q-heads, 4 kv-heads, D=128) on 8 trn2 cores.

Sharding: 16 (batch, kv-group) pairs -> 2 pairs/core; each pair carries 4 query
heads, so each core runs 8 independent causal-attention head-units.

Per head-unit math (all on one core, fp16 matmul dtypes):
  S^T[sk, sq] = K^T.T @ Q^T        (contraction over d=128 on partitions)
  P^T = exp(SCALE * S^T)           split across engines by k-tile row:
      rows 0,1,(4,5) -> ACT table exp (exact, fp16 out)
      rows 2,3,(6,7) -> DVE Schraudolph: int16 bits = s*C16 + BETA written
        through a bitcast view, giving 2^((bits-15360)/1024) ~ e^(s*SCALE)
        with ~2-3% zero-mean per-element error that cancels in the softmax
        ratio (the per-row engine split is safe because normalization divides
        per q-column; measured end-to-end error 4.2e-3 vs the 2e-2 gate).
  diagonal 128x128 blocks masked by a 0/1 multiply on Pool (gpsimd) -- the
    only elementwise engine with free cycles, and masks touch only SBUF
    (gpsimd cannot access PSUM, a hard walrus constraint).
  O[sq,d] = P^T.T @ V and den[sq] = P^T.T @ ones as separate contiguous
    accumulation groups into two [128,4,128] PSUM half-tiles + a [128,8] den
    tile (1 bank each; with 2x2-bank score buffers + 1-bank row-6/7 buffer
    this exactly fills the 8 PSUM banks).
  rec = 1/den (strided DVE reciprocal per half), out = O * rec as ONE
    broadcast tensor_mul per half (rec stride-0 expanded), fp16 out, DMA'd
    per head; the host casts to fp32.

Schedule per head u (PE stays dense; exp latency is hidden by interleaving
the previous head's PV between score rows, and the 5 main score-buffer uses
per head alternate the rotation parity so no buffer is reused before its
exp drains): r45, r67 | pv0(u-1)+recipA | r0, r3 | normA(u-1) | r2 | r1 |
pv1(u-1)+recipB | normB(u-1)+dma. Norms are emitted at chosen DVE-queue
positions: exp-r2 runs before normB so the diagonal mask m2 lands early
enough for the next head's PV, and recips stay right after their den groups
so the den-tile WAR never blocks the following head.
Rows 5 and 7 write only their causally-live columns; the paired exp reads
stale psum beyond them into pt slots that no PV matmul ever reads.
"""

import os
import sys

for _p in ("/opt/trn_rl_repo", "/root/.axon_site/_ro/trn_rl_repo"):
    if os.path.isdir(_p) and _p not in sys.path:
        sys.path.insert(0, _p)

import math
from contextlib import ExitStack

import numpy as np

import concourse.bass as bass
import concourse.tile as tile
from concourse import bacc, mybir
from concourse.bass_utils import run_bass_kernel_spmd

B = 4
S = 1024
H = 16
HKV = 4
G = H // HKV  # 4 query heads per kv head
D = 128
SCALE = 0.08838834764831845
NCORES = 8
PAIRS_PER_CORE = (B * HKV) // NCORES  # 2
NU = PAIRS_PER_CORE * G  # 8 head-units per core
NT = S // 128  # 8 tiles of 128 along seq
VW = D + 1  # V columns + ones column (fused softmax denominator)

C16 = 1024.0 * SCALE * math.log2(math.e)  # fp16 Schraudolph scale
BETA = 15301.0  # 15360 - 59 (centers the mantissa-linear hump under trunc)

FP16 = mybir.dt.float16
FP32 = mybir.dt.float32
I16 = mybir.dt.int16

_cache = {}


def build_program(n_units=NU):
    nc = bacc.Bacc("TRN2", target_bir_lowering=False, debug=False, num_devices=NCORES)

    qt_d = nc.dram_tensor("qt", [NU, D, S], FP16, kind="ExternalInput").ap()
    kt_d = nc.dram_tensor("kt", [PAIRS_PER_CORE, D, S], FP16, kind="ExternalInput").ap()
    vp_d = nc.dram_tensor("vp", [PAIRS_PER_CORE, NT, 128, VW], FP16, kind="ExternalInput").ap()
    mask_d = nc.dram_tensor("mask", [128, 128], FP16, kind="ExternalInput").ap()
    o_d = nc.dram_tensor("o", [PAIRS_PER_CORE, S, G, D], FP16, kind="ExternalOutput").ap()

    with tile.TileContext(nc) as tc, ExitStack() as ctx:
        const = ctx.enter_context(tc.tile_pool(name="const", bufs=1))
        pt_pool = ctx.enter_context(tc.tile_pool(name="pt_pool", bufs=2))
        small = ctx.enter_context(tc.tile_pool(name="small", bufs=2))
        outp = ctx.enter_context(tc.tile_pool(name="outp", bufs=2))
        psum = ctx.enter_context(tc.tile_pool(name="psum", bufs=2, space="PSUM"))
        psum1 = ctx.enter_context(tc.tile_pool(name="psum1", bufs=1, space="PSUM"))

        # ---- loads, ordered so head 0 can start as early as possible ----
        kt_sb = const.tile([128, PAIRS_PER_CORE, S], FP16)
        qt_sb = const.tile([128, NU, S], FP16)
        vp_sb = const.tile([128, PAIRS_PER_CORE * NT, VW], FP16)
        mask_sb = const.tile([128, 128], FP16)

        # load order matches first consumers: head 0 row 4/5 (kt cols 512:768,
        # qt cols 512:1024) starts after just two small DMAs
        nc.sync.dma_start(out=kt_sb[:, 0, 512:768], in_=kt_d[0][:, 512:768])
        nc.sync.dma_start(out=qt_sb[:, 0, 512:S], in_=qt_d[0][:, 512:S])
        nc.sync.dma_start(out=kt_sb[:, 0, 0:512], in_=kt_d[0][:, 0:512])
        nc.sync.dma_start(out=qt_sb[:, 0, 0:512], in_=qt_d[0][:, 0:512])
        nc.sync.dma_start(out=kt_sb[:, 0, 768:S], in_=kt_d[0][:, 768:S])
        nc.sync.dma_start(out=mask_sb, in_=mask_d)
        nc.sync.dma_start(
            out=vp_sb[:, 0:NT, :], in_=vp_d[0].rearrange("j r c -> r j c")
        )
        for u in range(1, G):
            nc.sync.dma_start(out=qt_sb[:, u, :], in_=qt_d[u])
        nc.sync.dma_start(out=kt_sb[:, 1, :], in_=kt_d[1])
        nc.sync.dma_start(
            out=vp_sb[:, NT : 2 * NT, :], in_=vp_d[1].rearrange("j r c -> r j c")
        )
        for u in range(G, NU):
            nc.sync.dma_start(out=qt_sb[:, u, :], in_=qt_d[u])

        state = {}  # per-head tiles carried into the next iteration

        def scores_row(u, pair, j, ps_flat):
            """Score matmuls for k-tile row j into a flat [128, 1024] psum view."""
            sq0 = 128 * j
            w = S - sq0
            lhsT = kt_sb[:, pair, sq0 : sq0 + 128]
            for c0 in range(0, w, 512):
                cw = min(512, w - c0)
                nc.tensor.matmul(
                    ps_flat[:, c0 : c0 + cw],
                    lhsT=lhsT,
                    rhs=qt_sb[:, u, sq0 + c0 : sq0 + c0 + cw],
                    start=True,
                    stop=True,
                )

        def exp_chunk(engine, out_ap, in_ap):
            if engine == "act":
                nc.scalar.activation(
                    out=out_ap,
                    in_=in_ap,
                    func=mybir.ActivationFunctionType.Exp,
                    scale=SCALE,
                )
            else:
                eng = nc.vector if engine == "dve" else nc.gpsimd
                eng.tensor_scalar(
                    out_ap.bitcast(I16),
                    in_ap,
                    C16,
                    BETA,
                    mybir.AluOpType.mult,
                    mybir.AluOpType.add,
                )

        def mask_row(pt, j):
            sq0 = 128 * j
            nc.gpsimd.tensor_mul(
                pt[:, j, sq0 : sq0 + 128], pt[:, j, sq0 : sq0 + 128], mask_sb
            )

        # exp engines: ACT r45,r0,r1; DVE r2; Pool r67,r3
        ENG = {"r45": "act", "r0": "act", "r1": "act", "r2": "dve",
               "r3": "dve", "r67": "dve"}

        def head_front(u):
            """Scores + exp + masks for head u; PV of head u-1 interleaved.
            Chunk order r45,r67 | pv0 | r0,r3 | pv1 | r1,r2 keeps PE dense and
            puts every exp ~2 chunks ahead of its psum-buffer reuse (5 main-tag
            uses per head alternate the rotation parity head to head)."""
            pair = u // G
            pt = pt_pool.tile([128, NT, S], FP16, tag="pt", name=f"pt_{u}")

            def ps_tile(name):
                return psum.tile([128, 2, 512], FP32, tag="ps", name=name)

            # --- rows 4,5 (paired, widened to 512) ---
            psA = ps_tile(f"ps45_{u}")
            for r in range(2):
                j = 4 + r
                sq0 = 128 * j
                nc.tensor.matmul(
                    psA[:, r, sq0 - 512 : 512],
                    lhsT=kt_sb[:, pair, sq0 : sq0 + 128],
                    rhs=qt_sb[:, u, sq0:S],
                    start=True,
                    stop=True,
                )
            exp_chunk(ENG["r45"], pt[:, 4:6, 512:S], psA[:, :, :])
            mask_row(pt, 4)
            mask_row(pt, 5)

            # --- rows 6,7 (own 1-bank tile; early so masks 6,7 never gate pv) ---
            psC = psum1.tile([128, 2, 256], FP32, tag="ps67", name=f"ps67_{u}")
            for r in range(2):
                j = 6 + r
                sq0 = 128 * j
                nc.tensor.matmul(
                    psC[:, r, sq0 - 768 : 256],
                    lhsT=kt_sb[:, pair, sq0 : sq0 + 128],
                    rhs=qt_sb[:, u, sq0:S],
                    start=True,
                    stop=True,
                )
            exp_chunk(ENG["r67"], pt[:, 6:8, 768:S], psC[:, :, :])
            mask_row(pt, 6)
            mask_row(pt, 7)

            # --- PV for previous head, first half (+recip+normalize) ---
            pv_half(u - 1, 0, 4)

            # --- row 0 ---
            psB = ps_tile(f"ps0_{u}")
            f0 = psB.rearrange("p a b -> p (a b)")
            scores_row(u, pair, 0, f0)
            exp_chunk(ENG["r0"], pt[:, 0, 0:S], f0[:, 0:S])
            mask_row(pt, 0)

            # --- row 3 ---
            psD = ps_tile(f"ps3_{u}")
            f3 = psD.rearrange("p a b -> p (a b)")
            scores_row(u, pair, 3, f3)
            exp_chunk(ENG["r3"], pt[:, 3, 384:S], f3[:, 0 : S - 384])
            mask_row(pt, 3)

            norm_half(u - 1, 0)

            # --- row 2 (DVE Schraudolph; masked immediately, same engine) ---
            psF = ps_tile(f"ps2_{u}")
            f2 = psF.rearrange("p a b -> p (a b)")
            scores_row(u, pair, 2, f2)
            exp_chunk(ENG["r2"], pt[:, 2, 256:S], f2[:, 0 : S - 256])
            mask_row(pt, 2)

            # --- row 1 ---
            psE = ps_tile(f"ps1_{u}")
            f1 = psE.rearrange("p a b -> p (a b)")
            scores_row(u, pair, 1, f1)
            exp_chunk(ENG["r1"], pt[:, 1, 128:S], f1[:, 0 : S - 128])
            mask_row(pt, 1)

            # --- PV for previous head, second half + store ---
            pv_half(u - 1, 4, 8)

            norm_half(u - 1, 1)
            finish_prev(u - 1)

            state[u] = pt

        def pv_half(u, i0, i1):
            if u < 0 or u >= n_units:
                return
            pair = u // G
            pt = state[u]
            half = i0 // 4
            pv = psum1.tile([128, 4, 128], FP32, tag=f"pv{half}", name=f"pv{half}_{u}")
            den = state.get(("den", u))
            if den is None:
                den = psum1.tile([128, NT], FP32, tag="den", name=f"den_{u}")
                state[("den", u)] = den
            for i in range(i0, i1):
                po = pv[:, i - i0, :]
                de = den[:, i : i + 1]
                for jj in range(i + 1):
                    nc.tensor.matmul(
                        po,
                        lhsT=pt[:, jj, 128 * i : 128 * i + 128],
                        rhs=vp_sb[:, pair * NT + jj, 0:D],
                        start=(jj == 0),
                        stop=(jj == i),
                    )
                for jj in range(i + 1):
                    nc.tensor.matmul(
                        de,
                        lhsT=pt[:, jj, 128 * i : 128 * i + 128],
                        rhs=vp_sb[:, pair * NT + jj, D:VW],
                        start=(jj == 0),
                        stop=(jj == i),
                    )
            # reciprocal right away; the normalize is emitted separately so
            # its DVE-queue position can be chosen (see norm_half)
            rec = small.tile([128, 4], FP32, tag=f"rec{half}", name=f"rec{half}_{u}")
            nc.vector.reciprocal_approx_fast(rec, den[:, i0:i1])
            state[("rec", u, half)] = (rec, pv)

        def norm_half(u, half):
            if u < 0 or u >= n_units:
                return
            rec, pv = state.pop(("rec", u, half))
            ob = state.get(("ob", u))
            if ob is None:
                ob = outp.tile([128, NT, D], FP16, tag="ot", name=f"ot_{u}")
                state[("ob", u)] = ob
            rb = rec.unsqueeze(2).broadcast_to([128, 4, D])
            nc.vector.tensor_mul(ob[:, 4 * half : 4 * half + 4, :], pv, rb)

        def finish_prev(u):
            if u < 0 or u >= n_units:
                return
            pair, h = divmod(u, G)
            state.pop(u)
            state.pop(("den", u), None)
            ob = state.pop(("ob", u))
            nc.sync.dma_start(
                out=o_d[pair, :, h, :].rearrange("(i s) d -> s i d", s=128), in_=ob
            )

        for u in range(n_units):
            head_front(u)
        pv_half(n_units - 1, 0, 4)
        norm_half(n_units - 1, 0)
        pv_half(n_units - 1, 4, 8)
        norm_half(n_units - 1, 1)
        finish_prev(n_units - 1)

    nc.compile()
    return nc


def _host_prep(q, k, v):
    """Build per-core input maps (shard + transpose + fp16 cast on host)."""
    q16 = np.ascontiguousarray(q.astype(np.float16))
    k16 = np.ascontiguousarray(k.astype(np.float16))
    v16 = np.ascontiguousarray(v.astype(np.float16))

    ii = np.arange(128)
    mask = (ii[None, :] >= ii[:, None]).astype(np.float16)  # [jj, ii]: ii >= jj

    in_maps = []
    for c in range(NCORES):
        qt = np.empty((NU, D, S), np.float16)
        kt = np.empty((PAIRS_PER_CORE, D, S), np.float16)
        vp = np.empty((PAIRS_PER_CORE, NT, 128, VW), np.float16)
        for p in range(PAIRS_PER_CORE):
            pg = c * PAIRS_PER_CORE + p
            b, g = divmod(pg, HKV)
            tok = slice(b * S, (b + 1) * S)
            for hh in range(G):
                qt[p * G + hh] = q16[tok, g * G + hh, :].T
            kt[p] = k16[tok, g, :].T
            vseg = v16[tok, g, :]  # [S, D]
            vp[p, :, :, :D] = vseg.reshape(NT, 128, D)
            vp[p, :, :, D] = np.float16(1.0)
        in_maps.append({"qt": qt, "kt": kt, "vp": vp, "mask": mask})
    return in_maps


def _gather(results):
    out = np.empty((B * S, H, D), np.float32)
    for c in range(NCORES):
        o = results[c]["o"]  # [PAIRS, S, G, D] fp16
        for p in range(PAIRS_PER_CORE):
            pg = c * PAIRS_PER_CORE + p
            b, g = divmod(pg, HKV)
            out[b * S : (b + 1) * S, g * G : (g + 1) * G, :] = o[p].astype(np.float32)
    return out


def kernel(q, k, v, cu_seqlens_q=None, cu_seqlens_k=None, **_ignored):
    if "nc" not in _cache:
        _cache["nc"] = build_program()
    nc = _cache["nc"]

    in_maps = _host_prep(np.asarray(q), np.asarray(k), np.asarray(v))
    res = run_bass_kernel_spmd(nc, in_maps, core_ids=list(range(NCORES)))
    return _gather(res.results)


# revision 57
# speedup vs baseline: 1.0180x; 1.0180x over previous
"""Causal GQA attention (B=4, S=1024, H=16 q-heads, 4 kv-heads, D=128) on 8 trn2 cores.

Sharding: 16 (batch, kv-group) pairs -> 2 pairs/core; each pair carries 4 query
heads, so each core runs 8 independent causal-attention head-units.

Per head-unit math (all on one core, fp16 matmul dtypes):
  S^T[sk, sq] = K^T.T @ Q^T        (contraction over d=128 on partitions)
  P^T = exp(SCALE * S^T)           split across engines by k-tile row:
      rows 0,1,(4,5) -> ACT table exp (exact, fp16 out)
      rows 2,3,(6,7) -> DVE Schraudolph: int16 bits = s*C16 + BETA written
        through a bitcast view, giving 2^((bits-15360)/1024) ~ e^(s*SCALE)
        with ~2-3% zero-mean per-element error that cancels in the softmax
        ratio (the per-row engine split is safe because normalization divides
        per q-column; measured end-to-end error 4.2e-3 vs the 2e-2 gate).
  diagonal 128x128 blocks masked by a 0/1 multiply on Pool (gpsimd) -- the
    only elementwise engine with free cycles, and masks touch only SBUF
    (gpsimd cannot access PSUM, a hard walrus constraint).
  O[sq,d] = P^T.T @ V and den[sq] = P^T.T @ ones as separate contiguous
    accumulation groups into two [128,4,128] PSUM half-tiles + a [128,8] den
    tile (1 bank each; with 2x2-bank score buffers + 1-bank row-6/7 buffer
    this exactly fills the 8 PSUM banks).
  rec = 1/den (strided DVE reciprocal per half), out = O * rec as ONE
    broadcast tensor_mul per half (rec stride-0 expanded), fp16 out, DMA'd
    per head; the host casts to fp32.

Schedule per head u (PE stays dense; exp latency is hidden by interleaving
the previous head's PV between score rows, and the 5 main score-buffer uses
per head alternate the rotation parity so no buffer is reused before its
exp drains): r45, r67 | pv0(u-1)+recipA | r0, r3 | normA(u-1) | r2 | r1 |
pv1(u-1)+recipB | normB(u-1)+dma. Norms are emitted at chosen DVE-queue
positions: exp-r2 runs before normB so the diagonal mask m2 lands early
enough for the next head's PV, and recips stay right after their den groups
so the den-tile WAR never blocks the following head.
Rows 5 and 7 write only their causally-live columns; the paired exp reads
stale psum beyond them into pt slots that no PV matmul ever reads.
"""

import os
import sys

for _p in ("/opt/trn_rl_repo", "/root/.axon_site/_ro/trn_rl_repo"):
    if os.path.isdir(_p) and _p not in sys.path:
        sys.path.insert(0, _p)

import math
from contextlib import ExitStack

import numpy as np

import concourse.bass as bass
import concourse.tile as tile
from concourse import bacc, mybir
from concourse.bass_utils import run_bass_kernel_spmd

B = 4
S = 1024
H = 16
HKV = 4
G = H // HKV  # 4 query heads per kv head
D = 128
SCALE = 0.08838834764831845
NCORES = 8
PAIRS_PER_CORE = (B * HKV) // NCORES  # 2
NU = PAIRS_PER_CORE * G  # 8 head-units per core
NT = S // 128  # 8 tiles of 128 along seq
VW = D + 1  # V columns + ones column (fused softmax denominator)

C16 = 1024.0 * SCALE * math.log2(math.e)  # fp16 Schraudolph scale
BETA = 15301.0  # 15360 - 59 (centers the mantissa-linear hump under trunc)

FP16 = mybir.dt.float16
FP32 = mybir.dt.float32
I16 = mybir.dt.int16

_cache = {}


def build_program(n_units=NU):
    nc = bacc.Bacc("TRN2", target_bir_lowering=False, debug=False, num_devices=NCORES)

    qt_d = nc.dram_tensor("qt", [NU, D, S], FP16, kind="ExternalInput").ap()
    kt_d = nc.dram_tensor("kt", [PAIRS_PER_CORE, D, S], FP16, kind="ExternalInput").ap()
    vp_d = nc.dram_tensor("vp", [PAIRS_PER_CORE, NT, 128, VW], FP16, kind="ExternalInput").ap()
    mask_d = nc.dram_tensor("mask", [128, 128], FP16, kind="ExternalInput").ap()
    o_d = nc.dram_tensor("o", [PAIRS_PER_CORE, S, G, D], FP16, kind="ExternalOutput").ap()

    with tile.TileContext(nc) as tc, ExitStack() as ctx:
        const = ctx.enter_context(tc.tile_pool(name="const", bufs=1))
        pt_pool = ctx.enter_context(tc.tile_pool(name="pt_pool", bufs=2))
        small = ctx.enter_context(tc.tile_pool(name="small", bufs=2))
        outp = ctx.enter_context(tc.tile_pool(name="outp", bufs=2))
        psum = ctx.enter_context(tc.tile_pool(name="psum", bufs=2, space="PSUM"))
        psum1 = ctx.enter_context(tc.tile_pool(name="psum1", bufs=1, space="PSUM"))

        # ---- loads, ordered so head 0 can start as early as possible ----
        kt_sb = const.tile([128, PAIRS_PER_CORE, S], FP16)
        qt_sb = const.tile([128, NU, S], FP16)
        vp_sb = const.tile([128, PAIRS_PER_CORE * NT, VW], FP16)
        mask_sb = const.tile([128, 128], FP16)

        # load order matches first consumers: head 0 row 4/5 (kt cols 512:768,
        # qt cols 512:1024) starts after just two small DMAs
        nc.sync.dma_start(out=kt_sb[:, 0, 512:768], in_=kt_d[0][:, 512:768])
        nc.sync.dma_start(out=qt_sb[:, 0, 512:S], in_=qt_d[0][:, 512:S])
        nc.sync.dma_start(out=kt_sb[:, 0, 0:512], in_=kt_d[0][:, 0:512])
        nc.sync.dma_start(out=qt_sb[:, 0, 0:512], in_=qt_d[0][:, 0:512])
        nc.sync.dma_start(out=kt_sb[:, 0, 768:S], in_=kt_d[0][:, 768:S])
        nc.sync.dma_start(out=mask_sb, in_=mask_d)
        nc.sync.dma_start(
            out=vp_sb[:, 0:NT, :], in_=vp_d[0].rearrange("j r c -> r j c")
        )
        for u in range(1, G):
            nc.sync.dma_start(out=qt_sb[:, u, :], in_=qt_d[u])
        nc.sync.dma_start(out=kt_sb[:, 1, :], in_=kt_d[1])
        nc.sync.dma_start(
            out=vp_sb[:, NT : 2 * NT, :], in_=vp_d[1].rearrange("j r c -> r j c")
        )
        for u in range(G, NU):
            nc.sync.dma_start(out=qt_sb[:, u, :], in_=qt_d[u])

        state = {}  # per-head tiles carried into the next iteration

        def scores_row(u, pair, j, ps_flat):
            """Score matmuls for k-tile row j into a flat [128, 1024] psum view."""
            sq0 = 128 * j
            w = S - sq0
            lhsT = kt_sb[:, pair, sq0 : sq0 + 128]
            for c0 in range(0, w, 512):
                cw = min(512, w - c0)
                nc.tensor.matmul(
                    ps_flat[:, c0 : c0 + cw],
                    lhsT=lhsT,
                    rhs=qt_sb[:, u, sq0 + c0 : sq0 + c0 + cw],
                    start=True,
                    stop=True,
                )

        def exp_chunk(engine, out_ap, in_ap):
            if engine == "act":
                nc.scalar.activation(
                    out=out_ap,
                    in_=in_ap,
                    func=mybir.ActivationFunctionType.Exp,
                    scale=SCALE,
                )
            else:
                eng = nc.vector if engine == "dve" else nc.gpsimd
                eng.tensor_scalar(
                    out_ap.bitcast(I16),
                    in_ap,
                    C16,
                    BETA,
                    mybir.AluOpType.mult,
                    mybir.AluOpType.add,
                )

        def mask_row(pt, j):
            sq0 = 128 * j
            nc.gpsimd.tensor_mul(
                pt[:, j, sq0 : sq0 + 128], pt[:, j, sq0 : sq0 + 128], mask_sb
            )

        # exp engines: ACT r45,r0,r1; DVE r2; Pool r67,r3
        ENG = {"r45": "act", "r0": "act", "r1": "act", "r2": "dve",
               "r3": "dve", "r67": "dve"}

        def head_front(u):
            """Scores + exp + masks for head u; PV of head u-1 interleaved.
            Chunk order r45,r67 | pv0 | r0,r3 | pv1 | r1,r2 keeps PE dense and
            puts every exp ~2 chunks ahead of its psum-buffer reuse (5 main-tag
            uses per head alternate the rotation parity head to head)."""
            pair = u // G
            pt = pt_pool.tile([128, NT, S], FP16, tag="pt", name=f"pt_{u}")

            def ps_tile(name):
                return psum.tile([128, 2, 512], FP32, tag="ps", name=name)

            # --- rows 4,5 (paired, widened to 512) ---
            psA = ps_tile(f"ps45_{u}")
            for r in range(2):
                j = 4 + r
                sq0 = 128 * j
                nc.tensor.matmul(
                    psA[:, r, sq0 - 512 : 512],
                    lhsT=kt_sb[:, pair, sq0 : sq0 + 128],
                    rhs=qt_sb[:, u, sq0:S],
                    start=True,
                    stop=True,
                )
            exp_chunk(ENG["r45"], pt[:, 4:6, 512:S], psA[:, :, :])
            mask_row(pt, 4)
            mask_row(pt, 5)

            # --- rows 6,7 (own 1-bank tile; early so masks 6,7 never gate pv) ---
            psC = psum1.tile([128, 2, 256], FP32, tag="ps67", name=f"ps67_{u}")
            for r in range(2):
                j = 6 + r
                sq0 = 128 * j
                nc.tensor.matmul(
                    psC[:, r, sq0 - 768 : 256],
                    lhsT=kt_sb[:, pair, sq0 : sq0 + 128],
                    rhs=qt_sb[:, u, sq0:S],
                    start=True,
                    stop=True,
                )
            exp_chunk(ENG["r67"], pt[:, 6:8, 768:S], psC[:, :, :])
            mask_row(pt, 6)
            mask_row(pt, 7)

            # --- PV for previous head, first half (+recip+normalize) ---
            pv_half(u - 1, 0, 4)

            # --- row 0 ---
            psB = ps_tile(f"ps0_{u}")
            f0 = psB.rearrange("p a b -> p (a b)")
            scores_row(u, pair, 0, f0)
            exp_chunk(ENG["r0"], pt[:, 0, 0:S], f0[:, 0:S])
            mask_row(pt, 0)

            # --- row 3 ---
            psD = ps_tile(f"ps3_{u}")
            f3 = psD.rearrange("p a b -> p (a b)")
            scores_row(u, pair, 3, f3)
            exp_chunk(ENG["r3"], pt[:, 3, 384:S], f3[:, 0 : S - 384])
            mask_row(pt, 3)

            norm_half(u - 1, 0)

            # --- row 2 (DVE Schraudolph; masked immediately, same engine) ---
            psF = ps_tile(f"ps2_{u}")
            f2 = psF.rearrange("p a b -> p (a b)")
            scores_row(u, pair, 2, f2)
            exp_chunk(ENG["r2"], pt[:, 2, 256:S], f2[:, 0 : S - 256])
            mask_row(pt, 2)

            # --- row 1 ---
            psE = ps_tile(f"ps1_{u}")
            f1 = psE.rearrange("p a b -> p (a b)")
            scores_row(u, pair, 1, f1)
            exp_chunk(ENG["r1"], pt[:, 1, 128:S], f1[:, 0 : S - 128])
            mask_row(pt, 1)

            # --- PV for previous head, second half + store ---
            pv_half(u - 1, 4, 8)

            norm_half(u - 1, 1)
            finish_prev(u - 1)

            state[u] = pt

        def pv_half(u, i0, i1):
            if u < 0 or u >= n_units:
                return
            pair = u // G
            pt = state[u]
            half = i0 // 4
            pv = psum1.tile([128, 4, 128], FP32, tag=f"pv{half}", name=f"pv{half}_{u}")
            den = state.get(("den", u))
            if den is None:
                den = psum1.tile([128, NT], FP32, tag="den", name=f"den_{u}")
                state[("den", u)] = den
            # group order by diag-mask availability: m1 (ACT exp-r1) lands
            # last, so tile i1 goes last and PE starts the block sooner
            order = [0, 3, 2, 1] if half == 0 else [4, 5, 6, 7]
            for i in order:
                po = pv[:, i - i0, :]
                for jj in range(i + 1):
                    nc.tensor.matmul(
                        po,
                        lhsT=pt[:, jj, 128 * i : 128 * i + 128],
                        rhs=vp_sb[:, pair * NT + jj, 0:D],
                        start=(jj == 0),
                        stop=(jj == i),
                    )
            for i in order:
                de = den[:, i : i + 1]
                for jj in range(i + 1):
                    nc.tensor.matmul(
                        de,
                        lhsT=pt[:, jj, 128 * i : 128 * i + 128],
                        rhs=vp_sb[:, pair * NT + jj, D:VW],
                        start=(jj == 0),
                        stop=(jj == i),
                    )
            # reciprocal right away; the normalize is emitted separately so
            # its DVE-queue position can be chosen (see norm_half)
            rec = small.tile([128, 4], FP32, tag=f"rec{half}", name=f"rec{half}_{u}")
            nc.vector.reciprocal_approx_fast(rec, den[:, i0:i1])
            state[("rec", u, half)] = (rec, pv)

        def norm_half(u, half):
            if u < 0 or u >= n_units:
                return
            rec, pv = state.pop(("rec", u, half))
            ob = state.get(("ob", u))
            if ob is None:
                ob = outp.tile([128, NT, D], FP16, tag="ot", name=f"ot_{u}")
                state[("ob", u)] = ob
            rb = rec.unsqueeze(2).broadcast_to([128, 4, D])
            nc.vector.tensor_mul(ob[:, 4 * half : 4 * half + 4, :], pv, rb)
            pair, h = divmod(u, G)
            nc.sync.dma_start(
                out=o_d[pair, 512 * half : 512 * half + 512, h, :].rearrange(
                    "(i s) d -> s i d", s=128
                ),
                in_=ob[:, 4 * half : 4 * half + 4, :],
            )

        def finish_prev(u):
            if u < 0 or u >= n_units:
                return
            state.pop(u)
            state.pop(("den", u), None)
            state.pop(("ob", u), None)

        for u in range(n_units):
            head_front(u)
        pv_half(n_units - 1, 0, 4)
        norm_half(n_units - 1, 0)
        pv_half(n_units - 1, 4, 8)
        norm_half(n_units - 1, 1)
        finish_prev(n_units - 1)

    nc.compile()
    return nc


def _host_prep(q, k, v):
    """Build per-core input maps (shard + transpose + fp16 cast on host)."""
    q16 = np.ascontiguousarray(q.astype(np.float16))
    k16 = np.ascontiguousarray(k.astype(np.float16))
    v16 = np.ascontiguousarray(v.astype(np.float16))

    ii = np.arange(128)
    mask = (ii[None, :] >= ii[:, None]).astype(np.float16)  # [jj, ii]: ii >= jj

    in_maps = []
    for c in range(NCORES):
        qt = np.empty((NU, D, S), np.float16)
        kt = np.empty((PAIRS_PER_CORE, D, S), np.float16)
        vp = np.empty((PAIRS_PER_CORE, NT, 128, VW), np.float16)
        for p in range(PAIRS_PER_CORE):
            pg = c * PAIRS_PER_CORE + p
            b, g = divmod(pg, HKV)
            tok = slice(b * S, (b + 1) * S)
            for hh in range(G):
                qt[p * G + hh] = q16[tok, g * G + hh, :].T
            kt[p] = k16[tok, g, :].T
            vseg = v16[tok, g, :]  # [S, D]
            vp[p, :, :, :D] = vseg.reshape(NT, 128, D)
            vp[p, :, :, D] = np.float16(1.0)
        in_maps.append({"qt": qt, "kt": kt, "vp": vp, "mask": mask})
    return in_maps


def _gather(results):
    out = np.empty((B * S, H, D), np.float32)
    for c in range(NCORES):
        o = results[c]["o"]  # [PAIRS, S, G, D] fp16
        for p in range(PAIRS_PER_CORE):
            pg = c * PAIRS_PER_CORE + p
            b, g = divmod(pg, HKV)
            out[b * S : (b + 1) * S, g * G : (g + 1) * G, :] = o[p].astype(np.float32)
    return out


def kernel(q, k, v, cu_seqlens_q=None, cu_seqlens_k=None, **_ignored):
    if "nc" not in _cache:
        _cache["nc"] = build_program()
    nc = _cache["nc"]

    in_maps = _host_prep(np.asarray(q), np.asarray(k), np.asarray(v))
    res = run_bass_kernel_spmd(nc, in_maps, core_ids=list(range(NCORES)))
    return _gather(res.results)


# revision 69
# speedup vs baseline: 1.0452x; 1.0267x over previous
"""Causal GQA attention (B=4, S=1024, H=16 q-heads, 4 kv-heads, D=128) on 8 trn2 cores.

Sharding: 16 (batch, kv-group) pairs -> 2 pairs/core; each pair carries 4 query
heads, so each core runs 8 independent causal-attention head-units.

Per head-unit math (all on one core, fp16 matmul dtypes):
  S^T[sk, sq] = K^T.T @ Q^T        (contraction over d=128 on partitions)
  P^T = exp(SCALE * S^T)           split across engines by k-tile row:
      rows 0,1,(4,5) -> ACT table exp (exact, fp16 out)
      rows 2,3,(6,7) -> DVE Schraudolph: int16 bits = s*C16 + BETA written
        through a bitcast view, giving 2^((bits-15360)/1024) ~ e^(s*SCALE)
        with ~2-3% zero-mean per-element error that cancels in the softmax
        ratio (the per-row engine split is safe because normalization divides
        per q-column; measured end-to-end error 4.2e-3 vs the 2e-2 gate).
  diagonal 128x128 blocks masked by a 0/1 multiply on Pool (gpsimd) -- the
    only elementwise engine with free cycles, and masks touch only SBUF
    (gpsimd cannot access PSUM, a hard walrus constraint).
  O[sq,d] = P^T.T @ V and den[sq] = P^T.T @ ones as separate contiguous
    accumulation groups into two [128,4,128] PSUM half-tiles + a [128,8] den
    tile (1 bank each; with 2x2-bank score buffers + 1-bank row-6/7 buffer
    this exactly fills the 8 PSUM banks).
  rec = 1/den (strided DVE reciprocal per half), out = O * rec as ONE
    broadcast tensor_mul per half (rec stride-0 expanded), fp16 out, DMA'd
    per head; the host casts to fp32.

Schedule per head u (PE stays dense; exp latency is hidden by interleaving
the previous head's PV between score rows, and the 5 main score-buffer uses
per head alternate the rotation parity so no buffer is reused before its
exp drains): r45, r67 | pv0(u-1)+recipA | r0, r3 | normA(u-1) | r2 | r1 |
pv1(u-1)+recipB | normB(u-1). Norms are emitted at chosen DVE-queue
positions: exp-r2 runs before normB so the diagonal mask m2 lands early
enough for the next head's PV, and recips stay right after their den groups
so the den-tile WAR never blocks the following head. Each norm half DMAs its
ob half immediately (overlaps the tail). Within pv halves, all O-groups run
before all den-groups, and tile groups go in diag-mask-readiness order
(i1 last: its mask follows the last ACT exp) so the in-order PE starts the
block as early as possible.
Rows 5 and 7 write only their causally-live columns; the paired exp reads
stale psum beyond them into pt slots that no PV matmul ever reads.
"""

import os
import sys

for _p in ("/opt/trn_rl_repo", "/root/.axon_site/_ro/trn_rl_repo"):
    if os.path.isdir(_p) and _p not in sys.path:
        sys.path.insert(0, _p)

import math
from contextlib import ExitStack

import numpy as np

import concourse.bass as bass
import concourse.tile as tile
from concourse import bacc, mybir
from concourse.bass_utils import run_bass_kernel_spmd

B = 4
S = 1024
H = 16
HKV = 4
G = H // HKV  # 4 query heads per kv head
D = 128
SCALE = 0.08838834764831845
NCORES = 8
PAIRS_PER_CORE = (B * HKV) // NCORES  # 2
NU = PAIRS_PER_CORE * G  # 8 head-units per core
NT = S // 128  # 8 tiles of 128 along seq
VW = D + 1  # V columns + ones column (fused softmax denominator)

C16 = 1024.0 * SCALE * math.log2(math.e)  # fp16 Schraudolph scale
BETA = 15301.0  # 15360 - 59 (centers the mantissa-linear hump under trunc)

FP16 = mybir.dt.float16
FP32 = mybir.dt.float32
I16 = mybir.dt.int16

_cache = {}


def build_program(n_units=NU):
    nc = bacc.Bacc("TRN2", target_bir_lowering=False, debug=False, num_devices=NCORES)

    qt_d = nc.dram_tensor("qt", [NU, D, S], FP16, kind="ExternalInput").ap()
    kt_d = nc.dram_tensor("kt", [PAIRS_PER_CORE, D, S], FP16, kind="ExternalInput").ap()
    vp_d = nc.dram_tensor("vp", [PAIRS_PER_CORE, NT, 128, VW], FP16, kind="ExternalInput").ap()
    mask_d = nc.dram_tensor("mask", [128, 128], FP16, kind="ExternalInput").ap()
    o_d = nc.dram_tensor("o", [PAIRS_PER_CORE, S, G, D], FP16, kind="ExternalOutput").ap()

    with tile.TileContext(nc) as tc, ExitStack() as ctx:
        const = ctx.enter_context(tc.tile_pool(name="const", bufs=1))
        pt_pool = ctx.enter_context(tc.tile_pool(name="pt_pool", bufs=2))
        small = ctx.enter_context(tc.tile_pool(name="small", bufs=2))
        outp = ctx.enter_context(tc.tile_pool(name="outp", bufs=2))
        psum = ctx.enter_context(tc.tile_pool(name="psum", bufs=2, space="PSUM"))
        psum1 = ctx.enter_context(tc.tile_pool(name="psum1", bufs=1, space="PSUM"))

        # ---- loads, ordered so head 0 can start as early as possible ----
        kt_sb = const.tile([128, PAIRS_PER_CORE, S], FP16)
        qt_sb = const.tile([128, NU, S], FP16)
        vp_sb = const.tile([128, PAIRS_PER_CORE * NT, VW], FP16)
        mask_sb = const.tile([128, 128], FP16)

        # load order matches first consumers: head 0 row 4/5 (kt cols 512:768,
        # qt cols 512:1024) starts after just two small DMAs
        nc.sync.dma_start(out=kt_sb[:, 0, 512:768], in_=kt_d[0][:, 512:768])
        nc.sync.dma_start(out=qt_sb[:, 0, 512:S], in_=qt_d[0][:, 512:S])
        nc.sync.dma_start(out=kt_sb[:, 0, 0:512], in_=kt_d[0][:, 0:512])
        nc.sync.dma_start(out=qt_sb[:, 0, 0:512], in_=qt_d[0][:, 0:512])
        nc.sync.dma_start(out=kt_sb[:, 0, 768:S], in_=kt_d[0][:, 768:S])
        nc.sync.dma_start(out=mask_sb, in_=mask_d)
        nc.sync.dma_start(
            out=vp_sb[:, 0:NT, :], in_=vp_d[0].rearrange("j r c -> r j c")
        )
        for u in range(1, G):
            nc.sync.dma_start(out=qt_sb[:, u, :], in_=qt_d[u])
        nc.sync.dma_start(out=kt_sb[:, 1, :], in_=kt_d[1])
        nc.sync.dma_start(
            out=vp_sb[:, NT : 2 * NT, :], in_=vp_d[1].rearrange("j r c -> r j c")
        )
        for u in range(G, NU):
            nc.sync.dma_start(out=qt_sb[:, u, :], in_=qt_d[u])

        state = {}  # per-head tiles carried into the next iteration

        def scores_row(u, pair, j, ps_flat):
            """Score matmuls for k-tile row j into a flat [128, 1024] psum view."""
            sq0 = 128 * j
            w = S - sq0
            lhsT = kt_sb[:, pair, sq0 : sq0 + 128]
            for c0 in range(0, w, 512):
                cw = min(512, w - c0)
                nc.tensor.matmul(
                    ps_flat[:, c0 : c0 + cw],
                    lhsT=lhsT,
                    rhs=qt_sb[:, u, sq0 + c0 : sq0 + c0 + cw],
                    start=True,
                    stop=True,
                )

        def exp_chunk(engine, out_ap, in_ap):
            if engine == "act":
                nc.scalar.activation(
                    out=out_ap,
                    in_=in_ap,
                    func=mybir.ActivationFunctionType.Exp,
                    scale=SCALE,
                )
            else:
                eng = nc.vector if engine == "dve" else nc.gpsimd
                eng.tensor_scalar(
                    out_ap.bitcast(I16),
                    in_ap,
                    C16,
                    BETA,
                    mybir.AluOpType.mult,
                    mybir.AluOpType.add,
                )

        def mask_row(pt, j):
            sq0 = 128 * j
            nc.gpsimd.tensor_mul(
                pt[:, j, sq0 : sq0 + 128], pt[:, j, sq0 : sq0 + 128], mask_sb
            )

        # exp engines: ACT r45,r0,r1; DVE r2; Pool r67,r3
        ENG = {"r45": "act", "r0": "act", "r1": "act", "r2": "dve",
               "r3": "dve", "r67": "dve"}

        def head_front(u, last=False):
            """Scores + exp + masks for head u; PV of head u-1 interleaved.
            For the last head, rows 0-3 go first so its own PV (the epilogue)
            overlaps its r45/r67 scores instead of serializing after them."""
            pair = u // G
            pt = pt_pool.tile([128, NT, S], FP16, tag="pt", name=f"pt_{u}")

            def ps_tile(name):
                return psum.tile([128, 2, 512], FP32, tag="ps", name=name)

            def c_r45():
                psA = ps_tile(f"ps45_{u}")
                for r in range(2):
                    j = 4 + r
                    sq0 = 128 * j
                    nc.tensor.matmul(
                        psA[:, r, sq0 - 512 : 512],
                        lhsT=kt_sb[:, pair, sq0 : sq0 + 128],
                        rhs=qt_sb[:, u, sq0:S],
                        start=True,
                        stop=True,
                    )
                exp_chunk(ENG["r45"], pt[:, 4:6, 512:S], psA[:, :, :])
                mask_row(pt, 4)
                mask_row(pt, 5)

            def c_r67():
                psC = psum1.tile([128, 2, 256], FP32, tag="ps67", name=f"ps67_{u}")
                for r in range(2):
                    j = 6 + r
                    sq0 = 128 * j
                    nc.tensor.matmul(
                        psC[:, r, sq0 - 768 : 256],
                        lhsT=kt_sb[:, pair, sq0 : sq0 + 128],
                        rhs=qt_sb[:, u, sq0:S],
                        start=True,
                        stop=True,
                    )
                exp_chunk(ENG["r67"], pt[:, 6:8, 768:S], psC[:, :, :])
                mask_row(pt, 6)
                mask_row(pt, 7)

            def c_row(j, eng_key):
                ps = ps_tile(f"ps{j}_{u}")
                f = ps.rearrange("p a b -> p (a b)")
                scores_row(u, pair, j, f)
                sq0 = 128 * j
                exp_chunk(ENG[eng_key], pt[:, j, sq0:S], f[:, 0 : S - sq0])
                mask_row(pt, j)

            if not last:
                c_r45()
                c_r67()
                pv_half(u - 1, 0, 4)
                c_row(0, "r0")
                c_row(3, "r3")
                norm_half(u - 1, 0)
                c_row(2, "r2")
                c_row(1, "r1")
                pv_half(u - 1, 4, 8)
                norm_half(u - 1, 1)
                finish_prev(u - 1)
            else:
                c_row(0, "r0")
                c_row(3, "r3")
                pv_half(u - 1, 0, 4)
                c_row(2, "r2")
                c_row(1, "r1")
                norm_half(u - 1, 0)
                c_r45()
                pv_half(u - 1, 4, 8)
                c_r67()
                norm_half(u - 1, 1)
                finish_prev(u - 1)

            state[u] = pt

        def pv_half(u, i0, i1):
            if u < 0 or u >= n_units:
                return
            pair = u // G
            pt = state[u]
            half = i0 // 4
            pv = psum1.tile([128, 4, 128], FP32, tag=f"pv{half}", name=f"pv{half}_{u}")
            den = state.get(("den", u))
            if den is None:
                den = psum1.tile([128, NT], FP32, tag="den", name=f"den_{u}")
                state[("den", u)] = den
            # group order by diag-mask availability: m1 (ACT exp-r1) lands
            # last, so tile i1 goes last and PE starts the block sooner
            order = [3, 0, 2, 1] if half == 0 else [4, 5, 6, 7]
            for i in order:
                po = pv[:, i - i0, :]
                for jj in range(i + 1):
                    nc.tensor.matmul(
                        po,
                        lhsT=pt[:, jj, 128 * i : 128 * i + 128],
                        rhs=vp_sb[:, pair * NT + jj, 0:D],
                        start=(jj == 0),
                        stop=(jj == i),
                    )
            for i in order:
                de = den[:, i : i + 1]
                for jj in range(i + 1):
                    nc.tensor.matmul(
                        de,
                        lhsT=pt[:, jj, 128 * i : 128 * i + 128],
                        rhs=vp_sb[:, pair * NT + jj, D:VW],
                        start=(jj == 0),
                        stop=(jj == i),
                    )
            # reciprocal right away; the normalize is emitted separately so
            # its DVE-queue position can be chosen (see norm_half)
            rec = small.tile([128, 4], FP32, tag=f"rec{half}", name=f"rec{half}_{u}")
            nc.vector.reciprocal_approx_fast(rec, den[:, i0:i1])
            state[("rec", u, half)] = (rec, pv)

        def norm_half(u, half):
            if u < 0 or u >= n_units:
                return
            rec, pv = state.pop(("rec", u, half))
            ob = state.get(("ob", u))
            if ob is None:
                ob = outp.tile([128, NT, D], FP16, tag="ot", name=f"ot_{u}")
                state[("ob", u)] = ob
            rb = rec.unsqueeze(2).broadcast_to([128, 4, D])
            nc.vector.tensor_mul(ob[:, 4 * half : 4 * half + 4, :], pv, rb)
            pair, h = divmod(u, G)
            nc.sync.dma_start(
                out=o_d[pair, 512 * half : 512 * half + 512, h, :].rearrange(
                    "(i s) d -> s i d", s=128
                ),
                in_=ob[:, 4 * half : 4 * half + 4, :],
            )

        def finish_prev(u):
            if u < 0 or u >= n_units:
                return
            state.pop(u)
            state.pop(("den", u), None)
            state.pop(("ob", u), None)

        for u in range(n_units):
            head_front(u, last=False)
        pv_half(n_units - 1, 0, 4)
        norm_half(n_units - 1, 0)
        pv_half(n_units - 1, 4, 8)
        norm_half(n_units - 1, 1)
        finish_prev(n_units - 1)

    nc.compile()
    return nc


def _host_prep(q, k, v):
    """Build per-core input maps (shard + transpose + fp16 cast on host)."""
    q16 = np.ascontiguousarray(q.astype(np.float16))
    k16 = np.ascontiguousarray(k.astype(np.float16))
    v16 = np.ascontiguousarray(v.astype(np.float16))

    ii = np.arange(128)
    mask = (ii[None, :] >= ii[:, None]).astype(np.float16)  # [jj, ii]: ii >= jj

    in_maps = []
    for c in range(NCORES):
        qt = np.empty((NU, D, S), np.float16)
        kt = np.empty((PAIRS_PER_CORE, D, S), np.float16)
        vp = np.empty((PAIRS_PER_CORE, NT, 128, VW), np.float16)
        for p in range(PAIRS_PER_CORE):
            pg = c * PAIRS_PER_CORE + p
            b, g = divmod(pg, HKV)
            tok = slice(b * S, (b + 1) * S)
            for hh in range(G):
                qt[p * G + hh] = q16[tok, g * G + hh, :].T
            kt[p] = k16[tok, g, :].T
            vseg = v16[tok, g, :]  # [S, D]
            vp[p, :, :, :D] = vseg.reshape(NT, 128, D)
            vp[p, :, :, D] = np.float16(1.0)
        in_maps.append({"qt": qt, "kt": kt, "vp": vp, "mask": mask})
    return in_maps


def _gather(results):
    out = np.empty((B * S, H, D), np.float32)
    for c in range(NCORES):
        o = results[c]["o"]  # [PAIRS, S, G, D] fp16
        for p in range(PAIRS_PER_CORE):
            pg = c * PAIRS_PER_CORE + p
            b, g = divmod(pg, HKV)
            out[b * S : (b + 1) * S, g * G : (g + 1) * G, :] = o[p].astype(np.float32)
    return out


def kernel(q, k, v, cu_seqlens_q=None, cu_seqlens_k=None, **_ignored):
    if "nc" not in _cache:
        _cache["nc"] = build_program()
    nc = _cache["nc"]

    in_maps = _host_prep(np.asarray(q), np.asarray(k), np.asarray(v))
    res = run_bass_kernel_spmd(nc, in_maps, core_ids=list(range(NCORES)))
    return _gather(res.results)
